# revision 8
# baseline (speedup 1.0000x reference)
"""Trainium2 Bass kernel for the EnhancedNavigationLTC model.

Pure data parallel over batch: 512 batch rows are sharded 8 ways (64 per
NeuronCore); all LTC/LN/FC parameters are replicated.  Inside each core the
computation is split into:

  phase 1  (parallel over all B*T):  sensory synapse reduction
           w_num_s / w_den_s  via per-u ACT sigmoid (scale/bias folded) and
           PE matmul accumulation over the 128 sensory inputs, written to a
           DRAM scratch laid out [t][b][feature] for cheap per-step reloads.

  phase 2  (sequential over T, 4 ODE unfolds per step):  recurrent synapses.
           The sigmoid argument  erev*sigma*v - erev*sigma*mu  is produced by
           a k=65 selector matmul (stationary = transposed state + ones row),
           sigmoid runs on ACT straight out of PSUM, and the two weighted
           reductions over the 64 units run on DVE (mult + windowed reduce).
           erev sign flips are folded into the sigmoid argument on the host:
             num = sum_u wp*s_hat - K,   den = sum_u wp*erev*s_hat + K.

  head:    output affine + LayerNorm + FC, all on-device.
"""

import numpy as np

U = 64
I = 128
O = 15
UNFOLDS = 4
EPS = 1e-8
LN_EPS = 1e-5
B_FULL, T_FULL = 512, 512
N_CORES = 8
B_CORE = B_FULL // N_CORES          # 64
TS_CHUNK = 16                       # timesteps per phase-1 chunk
CHUNK_COLS = TS_CHUNK * B_CORE      # 1024


def _softplus(x):
    return np.log1p(np.exp(-np.abs(x))) + np.maximum(x, 0.0)


def _host_consts(input_w, input_b, sensory_w, sensory_mu, sensory_sigma,
                 sensory_erev, w, mu, sigma, erev, gleak, vleak, cm,
                 output_w, output_b, ln_w, ln_b, fc_w, fc_b):
    """All parameter-derived device constants, as float32 numpy arrays."""
    f32 = np.float32
    wp = _softplus(w).astype(f32)                       # (U, U)  [u, v]
    swp = _softplus(sensory_w).astype(f32)              # (I, U)
    gleak_p = _softplus(gleak).astype(f32)              # (U,)
    cm_t = (_softplus(cm) * UNFOLDS).astype(f32)        # (U,)

    # ---- phase 1 (sensory) ----
    se = sensory_erev.astype(f32)                       # +-1  (I, U)
    ss = sensory_sigma.astype(f32)
    sens_scale = (se * ss * input_w[:, None]).astype(f32)               # (I, U)
    sens_bias = (se * ss * (input_b[:, None] - sensory_mu)).astype(f32)  # (I, U)
    Ks = (swp * (se < 0)).sum(axis=0).astype(f32)       # (U,)

    # zero-padded stationary weights: for u, cols (2u, 2u+1) hold
    # [swp[:, u], swp[:, u]*se[:, u]] so psum row 2u+k accumulates num/den.
    sens_lhsT = np.zeros((I, U, 2 * U), f32)
    for u in range(U):
        sens_lhsT[:, u, 2 * u] = swp[:, u]
        sens_lhsT[:, u, 2 * u + 1] = swp[:, u] * se[:, u]
    sens_lhsT = sens_lhsT.reshape(I, U * 2 * U)         # (128, 8192)

    # ---- phase 2 (recurrent) ----
    ee = erev.astype(f32)                               # +-1  (U, U) [u, v]
    esig = ee * sigma.astype(f32)
    esigmu = esig * mu.astype(f32)
    K = (wp * (ee < 0)).sum(axis=0).astype(f32)         # (V,)

    # streaming selector matrices: W_s (65, 2048); col f = vv*64 + u
    W = np.zeros((2, U + 1, 32 * U), f32)
    for s in range(2):
        for vv in range(32):
            v = 32 * s + vv
            for u in range(U):
                W[s, u, vv * U + u] = esig[u, v]
            W[s, U, vv * U:vv * U + U] = -esigmu[:, v]

    def repl_vu(mat):  # mat (U, U) [u, v] -> (128, 2048) [64s+b, vv*64+u]
        out = np.zeros((128, 32 * U), f32)
        for s in range(2):
            blk = mat[:, 32 * s:32 * s + 32].T.reshape(32 * U)   # vv-major
            out[64 * s:64 * s + 64, :] = blk[None, :]
        return out

    wpT_l = repl_vu(wp)
    wperevT_l = repl_vu(wp * ee)

    def repl_v(vec):   # vec (U,) [v] -> (128, 32) [64s+b, vv]
        out = np.zeros((128, 32), f32)
        for s in range(2):
            out[64 * s:64 * s + 64, :] = vec[32 * s:32 * s + 32][None, :]
        return out

    glvl = gleak_p * vleak.astype(f32)
    CN = -K + glvl - Ks            # added to num
    CD = cm_t + gleak_p + K + Ks + EPS   # added to den
    CN_l, CD_l = repl_v(CN), repl_v(CD)
    CNCD_l = np.zeros((128, 2 * 32), f32)
    CNCD_l[:, 0::2] = CN_l
    CNCD_l[:, 1::2] = CD_l
    cmt_l = repl_v(cm_t)

    # ---- head ----
    tile_b = lambda vec: np.tile(np.asarray(vec, f32)[None, :], (B_CORE, 1))
    consts = {
        "sens_scale": sens_scale, "sens_bias": sens_bias,
        "sens_lhsT": sens_lhsT,
        "W0": W[0], "W1": W[1],
        "wpT_l": wpT_l, "wperevT_l": wperevT_l,
        "CNCD_l": CNCD_l, "cmt_l": cmt_l,
        "ident": np.eye(128, dtype=f32),
        "ident2": np.vstack([np.eye(64, dtype=f32), np.eye(64, dtype=f32)]),
        "ow_l": tile_b(output_w), "ob_l": tile_b(output_b),
        "lng_l": tile_b(ln_w), "lnb_l": tile_b(ln_b),
        "fcwT": np.asarray(fc_w, f32).T.copy(),          # (U, O)
        "fcb_l": tile_b(fc_b),                           # (B_CORE, O)
    }
    return consts


def _build_program(T):
    """Build the per-core SPMD Bass program for T timesteps."""
    import concourse.bass as bass
    import concourse.tile as tile
    from concourse import bacc, mybir
    from contextlib import ExitStack

    F32 = mybir.dt.float32
    AF = mybir.ActivationFunctionType
    ALU = mybir.AluOpType
    AX = mybir.AxisListType

    n_chunks = T // TS_CHUNK
    nc = bacc.Bacc("TRN2", target_bir_lowering=False, debug=False)

    d_xT = nc.dram_tensor("xT", [I, T * B_CORE], F32, kind="ExternalInput")
    cshape = {
        "sens_scale": [I, U], "sens_bias": [I, U], "sens_lhsT": [I, U * 2 * U],
        "W0": [U + 1, 32 * U], "W1": [U + 1, 32 * U],
        "wpT_l": [128, 32 * U], "wperevT_l": [128, 32 * U],
        "CNCD_l": [128, 64], "cmt_l": [128, 32], "ident": [128, 128],
        "ident2": [128, 64],
        "ow_l": [B_CORE, U], "ob_l": [B_CORE, U],
        "lng_l": [B_CORE, U], "lnb_l": [B_CORE, U],
        "fcwT": [U, O], "fcb_l": [B_CORE, O],
    }
    d_c = {k: nc.dram_tensor(k, v, F32, kind="ExternalInput")
           for k, v in cshape.items()}
    d_out = nc.dram_tensor("out", [B_CORE, O], F32, kind="ExternalOutput")
    d_scr = nc.dram_tensor("scr", [T * B_CORE, 2 * U], F32)   # [t*64+b][f]

    with tile.TileContext(nc) as tc:
        with ExitStack() as ctx:
            cpool = ctx.enter_context(tc.tile_pool(name="consts", bufs=1))
            c = {}
            for k, shp in cshape.items():
                c[k] = cpool.tile(shp, F32, tag=k, name=k)
                nc.sync.dma_start(c[k][:], d_c[k][:])

            # =================== phase 1 ===================
            with ExitStack() as p1:
                pool1 = p1.enter_context(tc.tile_pool(name="p1", bufs=2))
                sgp = p1.enter_context(tc.tile_pool(name="p1sg", bufs=3))
                pps1 = p1.enter_context(
                    tc.tile_pool(name="p1ps", bufs=2, space="PSUM"))
                ppsT = p1.enter_context(
                    tc.tile_pool(name="p1psT", bufs=2, space="PSUM"))

                with tc.For_i(0, n_chunks * CHUNK_COLS, CHUNK_COLS) as iv:
                    xc = pool1.tile([I, CHUNK_COLS], F32, tag="xc")
                    nc.sync.dma_start(xc[:], d_xT[:, bass.ds(iv, CHUNK_COLS)])
                    ps_nd = pps1.tile([128, CHUNK_COLS], F32, tag="psnd")
                    for u in range(U):
                        sg = sgp.tile([I, CHUNK_COLS], F32, tag="sg")
                        nc.scalar.activation(
                            sg[:], xc[:], AF.Sigmoid,
                            bias=c["sens_bias"][:, u:u + 1],
                            scale=c["sens_scale"][:, u:u + 1])
                        for n in range(CHUNK_COLS // 512):
                            nc.tensor.matmul(
                                ps_nd[:, 512 * n:512 * n + 512],
                                c["sens_lhsT"][:, 128 * u:128 * u + 128],
                                sg[:, 512 * n:512 * n + 512],
                                start=(u == 0), stop=(u == U - 1))
                    ev = pool1.tile([128, CHUNK_COLS], F32, tag="ev")
                    nc.vector.tensor_copy(ev[:], ps_nd[:])
                    # transpose 128x128 blocks -> scratch [t][b][f]
                    for k in range(CHUNK_COLS // 128):
                        psT = ppsT.tile([128, 128], F32, tag="psT")
                        nc.tensor.transpose(
                            psT[:], ev[:, 128 * k:128 * k + 128], c["ident"][:])
                        w2 = pool1.tile([128, 128], F32, tag="w2")
                        nc.vector.tensor_copy(w2[:], psT[:])
                        nc.sync.dma_start(
                            d_scr[bass.ds(iv + 128 * k, 128), :], w2[:])

            # =================== phase 2 ===================
            with ExitStack() as p2:
                pool2 = p2.enter_context(tc.tile_pool(name="p2", bufs=2))
                spool = p2.enter_context(tc.tile_pool(name="p2state", bufs=1))
                pps2 = p2.enter_context(
                    tc.tile_pool(name="p2ps", bufs=1, space="PSUM"))
                ppsv = p2.enter_context(
                    tc.tile_pool(name="p2psv", bufs=1, space="PSUM"))

                v_cur = spool.tile([128, 32], F32, tag="v_cur")
                vstT = spool.tile([U + 1, U], F32, tag="vstT")
                nc.vector.memset(v_cur[:], 0.0)
                nc.vector.memset(vstT[0:U, :], 0.0)
                nc.vector.memset(vstT[U:U + 1, :], 1.0)

                with tc.For_i(0, T * B_CORE, B_CORE) as iv:
                    t_w = pool2.tile([128, 2 * 32], F32, tag="t_w")
                    nc.sync.dma_start(
                        t_w[:],
                        d_scr[bass.ds(iv, B_CORE), :]
                        .rearrange("b (s vk) -> s b vk", s=2))
                    wsum = pool2.tile([128, 2 * 32], F32, tag="wsum")
                    nc.vector.tensor_add(wsum[:], t_w[:], c["CNCD_l"][:])

                    for uf in range(UNFOLDS):
                        # pipeline the unfold in 512-col chunks so
                        # PE -> ACT -> DVE/GPSIMD overlap
                        ps_arg = pps2.tile([128, 32 * U], F32, tag="ps_arg")
                        sg2 = pool2.tile([128, 32 * U], F32, tag="sg2")
                        t1 = pool2.tile([128, 32 * U], F32, tag="t1")
                        t2 = pool2.tile([128, 32 * U], F32, tag="t2")
                        nump = pool2.tile([128, 32], F32, tag="nump")
                        denp = pool2.tile([128, 32], F32, tag="denp")
                        cmtv = pool2.tile([128, 32], F32, tag="cmtv")
                        nc.vector.tensor_mul(cmtv[:], v_cur[:], c["cmt_l"][:])
                        for n in range(4):
                            cs = slice(512 * n, 512 * n + 512)
                            rs = slice(8 * n, 8 * n + 8)
                            for s in range(2):
                                Wc = c["W0"] if s == 0 else c["W1"]
                                nc.tensor.matmul(
                                    ps_arg[64 * s:64 * s + 64, cs],
                                    vstT[:], Wc[:, cs],
                                    start=True, stop=True,
                                    tile_position=(0, 64 * s))
                            nc.scalar.activation(sg2[:, cs], ps_arg[:, cs],
                                                 AF.Sigmoid)
                            nc.vector.tensor_mul(t1[:, cs], sg2[:, cs],
                                                 c["wpT_l"][:, cs])
                            nc.gpsimd.tensor_mul(t2[:, cs], sg2[:, cs],
                                                 c["wperevT_l"][:, cs])
                            nc.vector.tensor_reduce(
                                nump[:, rs],
                                t1[:, cs].rearrange("p (v u) -> p v u", u=U),
                                axis=AX.X, op=ALU.add)
                            nc.vector.tensor_reduce(
                                denp[:, rs],
                                t2[:, cs].rearrange("p (v u) -> p v u", u=U),
                                axis=AX.X, op=ALU.add)

                        num1 = pool2.tile([128, 32], F32, tag="num1")
                        nc.vector.tensor_add(num1[:], nump[:], wsum[:, 0::2])
                        num2 = pool2.tile([128, 32], F32, tag="num2")
                        nc.vector.tensor_add(num2[:], num1[:], cmtv[:])
                        den1 = pool2.tile([128, 32], F32, tag="den1")
                        nc.vector.tensor_add(den1[:], denp[:], wsum[:, 1::2])
                        rec = pool2.tile([128, 32], F32, tag="rec")
                        nc.vector.reciprocal(rec[:], den1[:])
                        nc.vector.tensor_mul(v_cur[:], num2[:], rec[:])

                        psv0 = ppsv.tile([32, U], F32, tag="psv")
                        psv1 = ppsv.tile([32, U], F32, tag="psv")
                        nc.tensor.transpose(
                            psv0[:], v_cur[0:64, :], c["ident2"][0:64, :])
                        nc.tensor.transpose(
                            psv1[:], v_cur[64:128, :], c["ident2"][64:128, :])
                        nc.vector.tensor_copy(vstT[0:32, :], psv0[:])
                        nc.vector.tensor_copy(vstT[32:64, :], psv1[:])

                # =================== head ===================
                hp = ppsv.tile([B_CORE, U], F32, tag="psv")
                nc.tensor.transpose(hp[:], vstT[0:U, :], c["ident"][0:64, 0:64])
                h = pool2.tile([B_CORE, U], F32, tag="h")
                nc.vector.tensor_mul(h[:], hp[:], c["ow_l"][:])
                nc.vector.tensor_add(h[:], h[:], c["ob_l"][:])
                mean = pool2.tile([B_CORE, 1], F32, tag="mean")
                nc.vector.tensor_reduce(mean[:], h[:], axis=AX.X, op=ALU.add)
                nc.vector.tensor_scalar_mul(mean[:], mean[:], 1.0 / U)
                xc2 = pool2.tile([B_CORE, U], F32, tag="xc2")
                nc.vector.tensor_scalar(
                    xc2[:], h[:], mean[:], None, op0=ALU.subtract)
                sq = pool2.tile([B_CORE, U], F32, tag="sq")
                nc.vector.tensor_mul(sq[:], xc2[:], xc2[:])
                var = pool2.tile([B_CORE, 1], F32, tag="var")
                nc.vector.tensor_reduce(var[:], sq[:], axis=AX.X, op=ALU.add)
                nc.vector.tensor_scalar(
                    var[:], var[:], 1.0 / U, LN_EPS, op0=ALU.mult, op1=ALU.add)
                sd = pool2.tile([B_CORE, 1], F32, tag="sd")
                nc.scalar.sqrt(sd[:], var[:])
                rstd = pool2.tile([B_CORE, 1], F32, tag="rstd")
                nc.vector.reciprocal(rstd[:], sd[:])
                nc.vector.tensor_scalar(
                    xc2[:], xc2[:], rstd[:], None, op0=ALU.mult)
                nc.vector.tensor_mul(xc2[:], xc2[:], c["lng_l"][:])
                nc.vector.tensor_add(xc2[:], xc2[:], c["lnb_l"][:])
                hTp = ppsv.tile([U, B_CORE], F32, tag="psv")
                nc.tensor.transpose(hTp[:], xc2[:], c["ident"][0:64, 0:64])
                hT = pool2.tile([U, B_CORE], F32, tag="hT")
                nc.scalar.copy(hT[:], hTp[:])
                ps_fc = ppsv.tile([B_CORE, O], F32, tag="psv")
                nc.tensor.matmul(ps_fc[:], hT[:], c["fcwT"][:],
                                 start=True, stop=True)
                res = pool2.tile([B_CORE, O], F32, tag="res")
                nc.vector.tensor_add(res[:], ps_fc[:], c["fcb_l"][:])
                nc.sync.dma_start(d_out[:], res[:])

    nc.finalize()
    return nc


def _make_xT(x_core, T):
    """(B_CORE, T, I) -> (I, T*B_CORE) with col = chunk*1024 + ts*64 + b."""
    n_chunks = T // TS_CHUNK
    xt = x_core.reshape(B_CORE, n_chunks, TS_CHUNK, I)
    xt = np.ascontiguousarray(xt.transpose(3, 1, 2, 0))  # (I, c, ts, b)
    return xt.reshape(I, T * B_CORE).astype(np.float32)


_PROGRAM_CACHE = {}


def kernel(**inputs):
    import sys
    if '/opt/trn_rl_repo' not in sys.path:
        sys.path.insert(0, '/opt/trn_rl_repo')
    from concourse.bass_utils import run_bass_kernel_spmd

    x = np.asarray(inputs["x"], np.float32)
    B, T = x.shape[0], x.shape[1]
    consts = _host_consts(**{k: np.asarray(v) for k, v in inputs.items()
                             if k != "x"})

    if T not in _PROGRAM_CACHE:
        _PROGRAM_CACHE[T] = _build_program(T)
    nc = _PROGRAM_CACHE[T]

    in_maps = []
    for g in range(N_CORES):
        m = dict(consts)
        m["xT"] = _make_xT(x[g * B_CORE:(g + 1) * B_CORE], T)
        in_maps.append(m)
    res = run_bass_kernel_spmd(nc, in_maps, list(range(N_CORES)))
    return np.concatenate([res.results[g]["out"] for g in range(N_CORES)],
                          axis=0)
